# revision 1
# baseline (speedup 1.0000x reference)
"""Trainium2 Bass kernel for the Expected-Depth DP loss.

Computation (see reference):
  - edge_max = max over first 7 of 8 op-logits          [S, 64, 16]
  - w        = masked softmax over the 16-wide window   [S, 64, 16]
  - DP scan:  ed[j] = sum_k w[j,k] * (ed[base+k] + 1),  j = 2..65
              (base = max(j-16, 0); for j < 16 only the first j entries
              of the window are valid — the mask zeroes the rest)
  - loss     = sum_s theta[s] * softmax(beta[s]) . (ed[ii] + ed[jj])

Sharding: S=8192 stages split across 8 cores (pure data parallel,
1024 stages/core, processed as 8 partition-tiles of 128 stages).
Per-core partial losses are summed on the host.

Per-tile pipeline on a core:
  DVE: max-of-7 reduce, mask mult, window sums, normalize, DP steps,
       final beta dot products
  ACT: exp(edge_max), reciprocal via exp(-log(s)), exp(beta^T) fused with
       the PSUM->SBUF copy of the PE transpose
  PE : per-128-column transpose of beta, incidence matmul
       c[stage,k] = sum_e exp(beta)[stage,e] * M[k,e]  (M = edge incidence,
       extra all-ones column gives the softmax denominator for free),
       final cross-partition reduction
"""

import numpy as np

SW = 16          # DP window
NN = 64          # nodes per stage
NOPS = 8         # ops per edge (last excluded from the max)
S = 8192         # stages
E = 2016         # beta edges
P = 128          # SBUF partitions
N_CORES = 8
S_CORE = S // N_CORES        # 1024
T = S_CORE // P              # 8 stage-tiles per core
GRP = NN * SW                # 1024 floats of edge_max per stage
AW = GRP * NOPS              # 8192 floats of alpha per stage
EDW = 67                     # ed row stride (66 node slots + 1 pad)
NCH = 16                     # beta column chunks
ECH = E // NCH               # 126 edges per chunk
NMASK = 14                   # nodes with partially-valid windows

_CACHE = {}


def _host_consts():
    ii, jj = [], []
    for i in range(2, NN + 1):
        for j in range(i + 1, NN + 2):
            ii.append(i)
            jj.append(j)
    ii = np.asarray(ii)
    jj = np.asarray(jj)
    # incidence matrix chunks: mt[e_local, c*67 + k] = [ii==k] + [jj==k],
    # column 66 of each chunk is all ones (softmax denominator)
    mt = np.zeros((NCH, ECH, EDW), np.float32)
    for e in range(E):
        c, el = divmod(e, ECH)
        mt[c, el, ii[e]] += 1.0
        mt[c, el, jj[e]] += 1.0
        mt[c, el, EDW - 1] = 1.0
    import ml_dtypes

    mt = np.ascontiguousarray(
        mt.transpose(1, 0, 2).reshape(ECH, NCH * EDW)
    ).astype(ml_dtypes.bfloat16)
    # validity mask for the first 14 nodes (node n: rows k < n+2 valid)
    mask = np.zeros((NMASK, SW), np.float32)
    for n in range(NMASK):
        mask[n, : n + 2] = 1.0
    mask = np.ascontiguousarray(
        np.broadcast_to(mask.reshape(1, NMASK * SW), (P, NMASK * SW))
    )
    return mt, mask


def _build_nc():
    import concourse.bass as bass
    import concourse.mybir as mybir
    from concourse.tile import TileContext
    from concourse.vector_clock import ScopedClock, VectorClock

    # This walrus build rejects TPB instructions carrying more than one sem
    # wait (two for EventSemaphore), but Tile's wait assignment happily packs
    # 2-3. Split the extras onto single-wait NoOps on the same engine.
    if not getattr(TileContext, "_ant_wait_split", False):
        _orig_commit = TileContext._commit_instruction

        def _commit_split(self, inst, lazy_reg_writes=True):
            si = inst.sync_info
            limit = 2 if isinstance(inst, mybir.InstEventSemaphore) else 1
            if si is not None and si.on_wait and len(si.on_wait) > limit:
                waits = list(si.on_wait)
                for i, w in enumerate(waits[:-limit]):
                    nop = mybir.InstNoOp(
                        name=f"{inst.name}-sw{i}",
                        sync_info=mybir.SyncInfo(on_wait=[w], on_update=[]),
                        bass_nofuse=True,
                        engine=inst.engine,
                    )
                    _orig_commit(self, nop, lazy_reg_writes)
                inst.sync_info = mybir.SyncInfo(
                    on_wait=waits[-limit:], on_update=list(si.on_update)
                )
            return _orig_commit(self, inst, lazy_reg_writes)

        TileContext._commit_instruction = _commit_split
        TileContext._ant_wait_split = True

    # The stock TileContext tail drain packs every outstanding sem wait into
    # a single InstDrain; this walrus caps non-EventSemaphore instructions at
    # one wait. Emit one drain per outstanding semaphore instead.
    def _drain_and_barrier(self, tick_clock, wait_clock):
        nc = self.nc
        gc = tick_clock.global_clock
        n = len(gc)
        for i in range(n):
            t = gc[i]
            if t <= 0:
                continue
            vc = VectorClock([0] * n)
            vc.require_at_least(i, t)
            d = nc.sync.drain()
            wait_clock.add_sem_waits(d.ins, ScopedClock({None: vc}))
        nc.all_engine_barrier()
        assert self.sems is not None
        popped = nc._tile_sem_poison_stack.pop()
        assert popped is self._sem_poison
        nc.clear_and_free_semaphores(list(self.sems.allocated().values()))
        nc.all_engine_barrier()

    TileContext._drain_and_barrier = _drain_and_barrier

    f32 = mybir.dt.float32
    Alu = mybir.AluOpType
    Act = mybir.ActivationFunctionType
    X = mybir.AxisListType.X

    bf16 = mybir.dt.bfloat16
    nc = bass.Bass()
    alpha_d = nc.declare_dram_parameter("alpha_c", [S_CORE, AW], f32, isOutput=False)
    # beta pre-transposed on the host into chunk layout:
    # beta_t[el, t*2048 + c*128 + p] = beta[t*128 + p, c*126 + el]
    beta_d = nc.declare_dram_parameter("beta_t", [ECH, T * NCH * P], f32, isOutput=False)
    theta_d = nc.declare_dram_parameter("theta_t", [P, T], f32, isOutput=False)
    mask_d = nc.declare_dram_parameter("mask_c", [P, NMASK * SW], f32, isOutput=False)
    mt_d = nc.declare_dram_parameter("mt_c", [ECH, NCH * EDW], bf16, isOutput=False)
    out_d = nc.declare_dram_parameter("loss_part", [1, 1], f32, isOutput=True)

    with TileContext(nc) as tc:
        with (
            tc.tile_pool(name="consts", bufs=1) as cp,
            tc.tile_pool(name="alphap", bufs=2) as ap_pool,
            tc.tile_pool(name="mxp", bufs=2) as mxp,
            tc.tile_pool(name="persist", bufs=1) as pp,
            tc.tile_pool(name="smallp", bufs=4) as sp,
            tc.tile_pool(name="betap", bufs=2) as bp,
            tc.tile_pool(name="ebtp", bufs=2) as ep,
            tc.tile_pool(name="psc", bufs=2, space="PSUM") as psc,
        ):
            # issue the first alpha tile's DMA before anything else — it
            # gates the whole DVE pipeline
            a_first = ap_pool.tile([P, AW], f32, tag="a")
            nc.sync.dma_start(a_first[:, :], alpha_d[0:P, :])

            mask_sb = cp.tile([P, NMASK * SW], f32)
            nc.sync.dma_start(mask_sb[:, :], mask_d[:, :])
            mt_sb = cp.tile([ECH, NCH * EDW], bf16)
            nc.sync.dma_start(mt_sb[:, :], mt_d[:, :])
            theta_sb = cp.tile([P, T], f32)
            nc.sync.dma_start(theta_sb[:, :], theta_d[:, :])
            ones_sb = cp.tile([P, 1], f32)
            nc.vector.memset(ones_sb[:, :], 1.0)

            w_sb = pp.tile([P, T * GRP], f32)     # normalized softmax weights
            ed_sb = pp.tile([P, T * EDW], f32)    # DP state, zero-init
            tmp_sb = pp.tile([P, T * SW], f32)    # DP step scratch
            acc_sb = pp.tile([P, T], f32)         # per-tile theta*depth
            nc.vector.memset(ed_sb[:, :], 0.0)

            # ---- alpha phase: softmax weights per stage-tile ----
            for t in range(T):
                if t == 0:
                    a_t = a_first
                else:
                    a_t = ap_pool.tile([P, AW], f32, tag="a")
                    nc.sync.dma_start(a_t[:, :], alpha_d[t * P : (t + 1) * P, :])
                mx = mxp.tile([P, GRP], f32, tag="mx")
                nc.vector.reduce_max(
                    mx[:, :],
                    a_t.rearrange("p (g o) -> p g o", o=NOPS)[:, :, 0 : NOPS - 1],
                    axis=X,
                )
                e_sl = w_sb[:, t * GRP : (t + 1) * GRP]
                # softmax numerator without max-subtraction (|logits| <~ 6)
                nc.scalar.activation(e_sl, mx[:, :], Act.Exp)
                nc.vector.tensor_mul(
                    e_sl[:, 0 : NMASK * SW], e_sl[:, 0 : NMASK * SW], mask_sb[:, :]
                )
                s_t = sp.tile([P, NN], f32, tag="s")
                nc.vector.reduce_sum(
                    s_t[:, :], e_sl.rearrange("p (n k) -> p n k", k=SW), axis=X
                )
                lns = sp.tile([P, NN], f32, tag="lns")
                nc.scalar.activation(lns[:, :], s_t[:, :], Act.Ln)
                rs = sp.tile([P, NN], f32, tag="rs")
                nc.scalar.activation(rs[:, :], lns[:, :], Act.Exp, scale=-1.0)
                rs_b = rs.rearrange("p (n o) -> p n o", o=1).broadcast_to((P, NN, SW))
                e3 = e_sl.rearrange("p (n k) -> p n k", k=SW)
                nc.vector.tensor_mul(e3, e3, rs_b)

            # ---- DP phase: all 8 stage-tiles per step ----
            w4 = w_sb.rearrange("p (t n k) -> p t n k", t=T, k=SW)
            ed3 = ed_sb.rearrange("p (t k) -> p t k", t=T)
            tmp3 = tmp_sb.rearrange("p (t k) -> p t k", k=SW)
            for j in range(2, NN + 2):
                n = j - 2
                wid = min(j, SW)
                base = j - wid
                nc.vector.scalar_tensor_tensor(
                    tmp3[:, :, 0:wid],
                    ed3[:, :, base : base + wid],
                    1.0,
                    w4[:, :, n, 0:wid],
                    Alu.add,
                    Alu.mult,
                )
                nc.vector.reduce_sum(ed3[:, :, j : j + 1], tmp3[:, :, 0:wid], axis=X)

            # ---- beta phase ----
            # one PSUM tile holds all 8 c-blocks (2 banks) so every beta
            # matmul can run during the alpha/DP phases instead of
            # serializing behind the q-reads
            c_ps = psc.tile([P, T * EDW], f32, tag="c", bufs=1)
            for t in range(T):
                b_t = bp.tile([ECH, NCH * P], f32, tag="b")
                nc.sync.dma_start(
                    b_t[:, :], beta_d[:, t * NCH * P : (t + 1) * NCH * P]
                )
                eb_t = ep.tile([ECH, NCH * P], bf16, tag="eb")
                nc.scalar.activation(eb_t[:, :], b_t[:, :], Act.Exp)
                for c in range(NCH):
                    nc.tensor.matmul(
                        c_ps[:, t * EDW : (t + 1) * EDW],
                        eb_t[:, c * P : (c + 1) * P],
                        mt_sb[:, c * EDW : (c + 1) * EDW],
                        start=(c == 0),
                        stop=(c == NCH - 1),
                    )
            for t in range(T):
                prod = sp.tile([P, EDW - 1], f32, tag="prod")
                q = sp.tile([P, 1], f32, tag="q")
                nc.vector.scalar_tensor_tensor(
                    prod[:, :],
                    ed3[:, t, 0 : EDW - 1],
                    0.0,
                    c_ps[:, t * EDW : t * EDW + EDW - 1],
                    Alu.add,
                    Alu.mult,
                    accum_out=q[:, :],
                )
                rsb = sp.tile([P, 1], f32, tag="rsb")
                nc.vector.reciprocal(rsb[:, :], c_ps[:, t * EDW + EDW - 1 : t * EDW + EDW])
                nc.vector.scalar_tensor_tensor(
                    acc_sb[:, t : t + 1],
                    q[:, :],
                    rsb[:, :],
                    theta_sb[:, t : t + 1],
                    Alu.mult,
                    Alu.mult,
                )

            # ---- final reduction: 8 cols then 128 partitions ----
            accsum = sp.tile([P, 1], f32, tag="accsum")
            nc.vector.reduce_sum(accsum[:, :], acc_sb[:, :], axis=X)
            out_ps = psc.tile([1, 1], f32, tag="outp", bufs=1)
            nc.tensor.matmul(
                out_ps[:, :], accsum[:, :], ones_sb[:, :], start=True, stop=True
            )
            out_sb = sp.tile([1, 1], f32, tag="outs")
            nc.scalar.copy(out_sb[:, :], out_ps[:, :])
            nc.sync.dma_start(out_d[:, :], out_sb[:, :])

    return nc


def _get_compiled():
    if "nc" not in _CACHE:
        _CACHE["nc"] = _build_nc()
        _CACHE["consts"] = _host_consts()
    return _CACHE["nc"], _CACHE["consts"]


def _in_maps(alpha, beta, theta):
    mt, mask = _get_compiled()[1]
    alpha = np.ascontiguousarray(alpha, dtype=np.float32).reshape(S, AW)
    beta = np.ascontiguousarray(beta, dtype=np.float32)
    theta = np.ascontiguousarray(theta, dtype=np.float32)
    maps = []
    for c in range(N_CORES):
        sl = slice(c * S_CORE, (c + 1) * S_CORE)
        # [el, t*2048 + ch*128 + p] = beta[t*128 + p, ch*126 + el]
        beta_t = np.ascontiguousarray(
            beta[sl].reshape(T, P, NCH, ECH).transpose(3, 0, 2, 1).reshape(ECH, -1)
        )
        maps.append(
            {
                "alpha_c": alpha[sl],
                "beta_t": beta_t,
                "theta_t": np.ascontiguousarray(theta[sl].reshape(T, P).T),
                "mask_c": mask,
                "mt_c": mt,
            }
        )
    return maps


def _run(alpha, beta, theta, **spmd_kwargs):
    from concourse.bass_utils import run_bass_kernel_spmd

    nc, _ = _get_compiled()
    res = run_bass_kernel_spmd(
        nc, _in_maps(alpha, beta, theta), core_ids=list(range(N_CORES)), **spmd_kwargs
    )
    total = np.float32(0.0)
    for r in res.results:
        total += np.float32(r["loss_part"][0, 0])
    return np.float32(total), res


def kernel(alpha, beta, theta):
    out, _ = _run(alpha, beta, theta)
    return out



# revision 2
# speedup vs baseline: 1.2523x; 1.2523x over previous
"""Trainium2 Bass kernel for the Expected-Depth DP loss.

Computation (see reference):
  - edge_max = max over first 7 of 8 op-logits          [S, 64, 16]
  - w        = masked softmax over the 16-wide window   [S, 64, 16]
  - DP scan:  ed[j] = sum_k w[j,k] * (ed[base+k] + 1),  j = 2..65
  - loss     = sum_s theta[s] * softmax(beta[s]) . (ed[ii] + ed[jj])

Sharding: S=8192 stages split across 8 cores (pure data parallel,
1024 stages/core as 128 partitions x 8 free slots). Per-core partial
losses are summed on the host.

Layout tricks vs the v0 kernel:
  - alpha is staged in HBM as 7 op-major bf16 planes (op 7 is unused by
    the reference and never shipped), grouped by 16-node chunks. The
    max-of-7 becomes 6 contiguous bf16 tensor_tensor max ops at DVE 2x
    mode instead of one giant 1x tensor_reduce.
  - everything alpha/beta-sized is bf16 (half the HBM traffic, 2x DVE).
  - the kernel pipelines by node group: tree/exp/softmax/DP for nodes
    [16g, 16g+16) run while the next group's planes stream in.
  - beta DMAs ride the scalar-engine HWDGE ring so they don't queue
    behind the alpha planes on the sync ring.
"""

import numpy as np

SW = 16          # DP window
NN = 64          # nodes per stage
S = 8192         # stages
E = 2016         # beta edges
P = 128          # SBUF partitions
N_CORES = 8
S_CORE = S // N_CORES        # 1024
T = S_CORE // P              # 8 stage slots per partition
NG = 4                       # node groups
GN = NN // NG                # 16 nodes per group
GW = GN * SW                 # 256 floats of edge_max per stage per group
GF = T * GW                  # 2048 free elems per group tile
NPL = 7                      # op planes
EDW = 67                     # ed row stride (66 node slots + 1 pad)
NCH = 16                     # beta column chunks
ECH = E // NCH               # 126 edges per chunk
NMASK = 14                   # nodes with partially-valid windows

_CACHE = {}


def _host_consts():
    import ml_dtypes

    ii, jj = [], []
    for i in range(2, NN + 1):
        for j in range(i + 1, NN + 2):
            ii.append(i)
            jj.append(j)
    ii = np.asarray(ii)
    jj = np.asarray(jj)
    # incidence matrix chunks: mt[e_local, c*67 + k] = [ii==k] + [jj==k],
    # column 66 of each chunk is all ones (softmax denominator)
    mt = np.zeros((NCH, ECH, EDW), np.float32)
    for e in range(E):
        c, el = divmod(e, ECH)
        mt[c, el, ii[e]] += 1.0
        mt[c, el, jj[e]] += 1.0
        mt[c, el, EDW - 1] = 1.0
    mt = np.ascontiguousarray(
        mt.transpose(1, 0, 2).reshape(ECH, NCH * EDW)
    ).astype(ml_dtypes.bfloat16)
    # validity mask for the first 14 nodes (node n: rows k < n+2 valid)
    mask = np.zeros((NMASK, SW), np.float32)
    for n in range(NMASK):
        mask[n, : n + 2] = 1.0
    mask = np.ascontiguousarray(
        np.broadcast_to(mask.reshape(1, NMASK * SW), (P, NMASK * SW))
    ).astype(ml_dtypes.bfloat16)
    return mt, mask


def _build_nc():
    import concourse.bass as bass
    import concourse.mybir as mybir
    from concourse.tile import TileContext
    from concourse.vector_clock import ScopedClock, VectorClock

    # This walrus build rejects TPB instructions carrying more than one sem
    # wait (two for EventSemaphore), but Tile's wait assignment happily packs
    # 2-3. Split the extras onto single-wait NoOps on the same engine.
    if not getattr(TileContext, "_ant_wait_split", False):
        _orig_commit = TileContext._commit_instruction

        def _commit_split(self, inst, lazy_reg_writes=True):
            si = inst.sync_info
            limit = 2 if isinstance(inst, mybir.InstEventSemaphore) else 1
            if si is not None and si.on_wait and len(si.on_wait) > limit:
                waits = list(si.on_wait)
                for i, w in enumerate(waits[:-limit]):
                    nop = mybir.InstNoOp(
                        name=f"{inst.name}-sw{i}",
                        sync_info=mybir.SyncInfo(on_wait=[w], on_update=[]),
                        bass_nofuse=True,
                        engine=inst.engine,
                    )
                    _orig_commit(self, nop, lazy_reg_writes)
                inst.sync_info = mybir.SyncInfo(
                    on_wait=waits[-limit:], on_update=list(si.on_update)
                )
            return _orig_commit(self, inst, lazy_reg_writes)

        TileContext._commit_instruction = _commit_split
        TileContext._ant_wait_split = True

    # The stock TileContext tail drain packs every outstanding sem wait into
    # a single InstDrain; this walrus caps non-EventSemaphore instructions at
    # one wait. Emit one drain per outstanding semaphore instead.
    def _drain_and_barrier(self, tick_clock, wait_clock):
        nc = self.nc
        gc = tick_clock.global_clock
        n = len(gc)
        for i in range(n):
            t = gc[i]
            if t <= 0:
                continue
            vc = VectorClock([0] * n)
            vc.require_at_least(i, t)
            d = nc.sync.drain()
            wait_clock.add_sem_waits(d.ins, ScopedClock({None: vc}))
        nc.all_engine_barrier()
        assert self.sems is not None
        popped = nc._tile_sem_poison_stack.pop()
        assert popped is self._sem_poison
        nc.clear_and_free_semaphores(list(self.sems.allocated().values()))
        nc.all_engine_barrier()

    f32 = mybir.dt.float32
    bf16 = mybir.dt.bfloat16
    Alu = mybir.AluOpType
    Act = mybir.ActivationFunctionType
    X = mybir.AxisListType

    TileContext._drain_and_barrier = _drain_and_barrier

    nc = bass.Bass()
    # alpha planes: row g*128+p, free [o(7), t(8), nl(16), k(16)]
    alpha_d = nc.declare_dram_parameter(
        "alpha_p", [NG * P, NPL * GF], bf16, isOutput=False
    )
    # beta pre-transposed on the host into chunk layout:
    # beta_t[el, t*2048 + c*128 + p] = beta[t*128 + p, c*126 + el]
    beta_d = nc.declare_dram_parameter("beta_t", [ECH, T * NCH * P], bf16, isOutput=False)
    theta_d = nc.declare_dram_parameter("theta_t", [P, T], f32, isOutput=False)
    mask_d = nc.declare_dram_parameter("mask_c", [P, NMASK * SW], bf16, isOutput=False)
    mt_d = nc.declare_dram_parameter("mt_c", [ECH, NCH * EDW], bf16, isOutput=False)
    out_d = nc.declare_dram_parameter("loss_part", [1, 1], f32, isOutput=True)

    with TileContext(nc) as tc:
        with (
            tc.tile_pool(name="consts", bufs=1) as cp,
            tc.tile_pool(name="planes", bufs=2) as plp,
            tc.tile_pool(name="tree", bufs=2) as trp,
            tc.tile_pool(name="persist", bufs=1) as pp,
            tc.tile_pool(name="smallp", bufs=4) as sp,
            tc.tile_pool(name="betap", bufs=2) as bp,
            tc.tile_pool(name="ebtp", bufs=2) as ep,
            tc.tile_pool(name="psc", bufs=2, space="PSUM") as psc,
        ):
            # first plane-group DMA gates the whole DVE pipeline
            pl0 = plp.tile([P, NPL * GF], bf16, tag="pl")
            nc.sync.dma_start(pl0[:, :], alpha_d[0:P, :])

            # consts + beta ride the scalar-engine HWDGE ring
            mask_sb = cp.tile([P, NMASK * SW], bf16)
            nc.scalar.dma_start(mask_sb[:, :], mask_d[:, :])
            mt_sb = cp.tile([ECH, NCH * EDW], bf16)
            nc.scalar.dma_start(mt_sb[:, :], mt_d[:, :])
            theta_sb = cp.tile([P, T], f32)
            nc.scalar.dma_start(theta_sb[:, :], theta_d[:, :])
            ones_sb = cp.tile([P, 1], f32)
            nc.vector.memset(ones_sb[:, :], 1.0)

            w_sb = pp.tile([P, NG * GF], bf16)    # softmax weights, grouped
            ed_sb = pp.tile([P, T * EDW], f32)    # DP state, zero-init
            tmp_sb = pp.tile([P, T * SW], f32)    # DP step scratch
            acc_sb = pp.tile([P, T], f32)         # per-slot theta*depth
            nc.vector.memset(ed_sb[:, :], 0.0)

            ed3 = ed_sb.rearrange("p (t k) -> p t k", t=T)
            tmp3 = tmp_sb.rearrange("p (t k) -> p t k", k=SW)

            # beta matmuls accumulate into one PSUM tile (2 banks) so they
            # can run under the alpha phase
            c_ps = psc.tile([P, T * EDW], f32, tag="c", bufs=1)

            def beta_tile(t):
                b_t = bp.tile([ECH, NCH * P], bf16, tag="b")
                nc.scalar.dma_start(
                    b_t[:, :], beta_d[:, t * NCH * P : (t + 1) * NCH * P]
                )
                eb_t = ep.tile([ECH, NCH * P], bf16, tag="eb")
                nc.scalar.activation(eb_t[:, :], b_t[:, :], Act.Exp)
                for c in range(NCH):
                    nc.tensor.matmul(
                        c_ps[:, t * EDW : (t + 1) * EDW],
                        eb_t[:, c * P : (c + 1) * P],
                        mt_sb[:, c * EDW : (c + 1) * EDW],
                        start=(c == 0),
                        stop=(c == NCH - 1),
                    )

            for g in range(NG):
                if g == 0:
                    pl = pl0
                else:
                    pl = plp.tile([P, NPL * GF], bf16, tag="pl")
                    nc.sync.dma_start(pl[:, :], alpha_d[g * P : (g + 1) * P, :])
                pv = pl.rearrange("p (o f) -> p o f", o=NPL)

                # 6-op max tree, all operands contiguous bf16 (2x mode)
                la = trp.tile([P, GF], bf16, tag="la")
                lb = trp.tile([P, GF], bf16, tag="lb")
                nc.vector.tensor_tensor(la[:, :], pv[:, 0, :], pv[:, 1, :], Alu.max)
                nc.vector.tensor_tensor(lb[:, :], pv[:, 2, :], pv[:, 3, :], Alu.max)
                nc.vector.tensor_tensor(la[:, :], la[:, :], lb[:, :], Alu.max)
                nc.vector.tensor_tensor(lb[:, :], pv[:, 4, :], pv[:, 5, :], Alu.max)
                nc.vector.tensor_tensor(lb[:, :], lb[:, :], pv[:, 6, :], Alu.max)
                mxg = trp.tile([P, GF], bf16, tag="mx")
                nc.vector.tensor_tensor(mxg[:, :], la[:, :], lb[:, :], Alu.max)

                # softmax numerator without max-subtraction (|logits| <~ 6)
                e_sl = w_sb[:, g * GF : (g + 1) * GF]
                nc.scalar.activation(e_sl, mxg[:, :], Act.Exp)
                if g == 0:
                    # zero the invalid window slots of nodes 0..13
                    e30 = e_sl.rearrange("p (t m) -> p t m", m=GW)[
                        :, :, 0 : NMASK * SW
                    ]
                    mask_b = mask_sb.rearrange(
                        "p (o m) -> p o m", o=1
                    ).broadcast_to((P, T, NMASK * SW))
                    nc.vector.tensor_mul(e30, e30, mask_b)

                s_g = sp.tile([P, T * GN], f32, tag="s")
                nc.vector.reduce_sum(
                    s_g[:, :], e_sl.rearrange("p (n k) -> p n k", k=SW), axis=X.X
                )
                lns = sp.tile([P, T * GN], f32, tag="lns")
                nc.scalar.activation(lns[:, :], s_g[:, :], Act.Ln)
                rs = sp.tile([P, T * GN], bf16, tag="rs")
                nc.scalar.activation(rs[:, :], lns[:, :], Act.Exp, scale=-1.0)
                e3 = e_sl.rearrange("p (n k) -> p n k", k=SW)
                rs_b = rs.rearrange("p (n o) -> p n o", o=1).broadcast_to(
                    (P, T * GN, SW)
                )
                nc.vector.tensor_mul(e3, e3, rs_b)

                # two beta tiles' worth of work per alpha group
                beta_tile(2 * g)
                beta_tile(2 * g + 1)

                # DP steps for this group's nodes (all 8 stage slots at once)
                wg = w_sb[:, g * GF : (g + 1) * GF].rearrange(
                    "p (t n k) -> p t n k", t=T, k=SW
                )
                for nl in range(GN):
                    j = g * GN + nl + 2
                    wid = min(j, SW)
                    base = j - wid
                    nc.vector.scalar_tensor_tensor(
                        tmp3[:, :, 0:wid],
                        ed3[:, :, base : base + wid],
                        1.0,
                        wg[:, :, nl, 0:wid],
                        Alu.add,
                        Alu.mult,
                    )
                    nc.vector.reduce_sum(
                        ed3[:, :, j : j + 1], tmp3[:, :, 0:wid], axis=X.X
                    )

            # ---- final dots ----
            for t in range(T):
                prod = sp.tile([P, EDW - 1], f32, tag="prod")
                q = sp.tile([P, 1], f32, tag="q")
                nc.vector.scalar_tensor_tensor(
                    prod[:, :],
                    ed3[:, t, 0 : EDW - 1],
                    0.0,
                    c_ps[:, t * EDW : t * EDW + EDW - 1],
                    Alu.add,
                    Alu.mult,
                    accum_out=q[:, :],
                )
                rsb = sp.tile([P, 1], f32, tag="rsb")
                nc.vector.reciprocal(
                    rsb[:, :], c_ps[:, t * EDW + EDW - 1 : t * EDW + EDW]
                )
                nc.vector.scalar_tensor_tensor(
                    acc_sb[:, t : t + 1],
                    q[:, :],
                    rsb[:, :],
                    theta_sb[:, t : t + 1],
                    Alu.mult,
                    Alu.mult,
                )

            # ---- final reduction: 8 cols then 128 partitions ----
            accsum = sp.tile([P, 1], f32, tag="accsum")
            nc.vector.reduce_sum(accsum[:, :], acc_sb[:, :], axis=X.X)
            out_ps = psc.tile([1, 1], f32, tag="outp", bufs=1)
            nc.tensor.matmul(
                out_ps[:, :], accsum[:, :], ones_sb[:, :], start=True, stop=True
            )
            out_sb = sp.tile([1, 1], f32, tag="outs")
            nc.scalar.copy(out_sb[:, :], out_ps[:, :])
            nc.sync.dma_start(out_d[:, :], out_sb[:, :])

    return nc


def _get_compiled():
    if "nc" not in _CACHE:
        _CACHE["nc"] = _build_nc()
        _CACHE["consts"] = _host_consts()
    return _CACHE["nc"], _CACHE["consts"]


def _in_maps(alpha, beta, theta):
    import ml_dtypes

    mt, mask = _get_compiled()[1]
    alpha = np.ascontiguousarray(alpha, dtype=np.float32)
    beta = np.ascontiguousarray(beta, dtype=np.float32)
    theta = np.ascontiguousarray(theta, dtype=np.float32)
    alpha_bf = alpha.astype(ml_dtypes.bfloat16)
    beta_bf = beta.astype(ml_dtypes.bfloat16)
    maps = []
    for c in range(N_CORES):
        sl = slice(c * S_CORE, (c + 1) * S_CORE)
        # [t, p, g, nl, k, o] -> [g, p, o, t, nl, k], drop op 7
        A = alpha_bf[sl].reshape(T, P, NG, GN, SW, 8)
        planes = np.ascontiguousarray(A.transpose(2, 1, 5, 0, 3, 4)[:, :, :NPL])
        # [el, t*2048 + ch*128 + p] = beta[t*128 + p, ch*126 + el]
        beta_t = np.ascontiguousarray(
            beta_bf[sl].reshape(T, P, NCH, ECH).transpose(3, 0, 2, 1).reshape(ECH, -1)
        )
        maps.append(
            {
                "alpha_p": planes.reshape(NG * P, NPL * GF),
                "beta_t": beta_t,
                "theta_t": np.ascontiguousarray(theta[sl].reshape(T, P).T),
                "mask_c": mask,
                "mt_c": mt,
            }
        )
    return maps


def _run(alpha, beta, theta, **spmd_kwargs):
    from concourse.bass_utils import run_bass_kernel_spmd

    nc, _ = _get_compiled()
    res = run_bass_kernel_spmd(
        nc, _in_maps(alpha, beta, theta), core_ids=list(range(N_CORES)), **spmd_kwargs
    )
    total = np.float32(0.0)
    for r in res.results:
        total += np.float32(r["loss_part"][0, 0])
    return np.float32(total), res


def kernel(alpha, beta, theta):
    out, _ = _run(alpha, beta, theta)
    return out
